# revision 37
# baseline (speedup 1.0000x reference)
"""GAT-pooling segment-softmax kernel for 8 Trainium2 NeuronCores.

Math (matches the reference):
    feats  = relu(x @ W1.T + b1)          [E, H]
    scores = feats @ w2 + b2              [E]
    w      = segment_softmax(scores)      (per segment of sorted batch_indices)
    out[s] = sum_{i in seg s} w_i * x_i   [S, H]

Scores are O(1) here, so exp() without the per-segment max subtraction is
numerically safe; softmax normalization happens on the host from per-segment
partial sums.

Device layout (per core, SPMD — one program, per-core data). Edges are
padded per-core to G supermacros of 4096 edges; edge = g*4096 + p*32 + q
(p = partition, q = subchunk). The host ships TWO layouts of x:
  * xb2 [G, 128, QPG*(H+1)]: subchunk-major rows with a constant-1 column
    appended per subchunk (for the fused denominator), ~8KB/partition
    contiguous DMA.
  * xt  [G, 128, QPG*128]: the per-supermacro transpose (h on partitions,
    e_local = q*128 + p on free) in fp8_e4m3 — enough fidelity for the
    attention-score path — so the feats matmul streams it directly: no PE
    transposes, no PSUM->SBUF copy on DVE, and half the DMA bytes.

Per supermacro g (software-pipelined one stage deep so PE never waits on
ACT):
  * feats.T = W1 @ x.T: 8 matmuls, w1t stationary, xt streamed N=512
  * ACT relu(+b1) -> fr [ho, 4096] bf16
  * scores col q = fr_q.T @ w2 ([128, 1] psum); ACT exp -> e [128, QPG]
  * DVE builds A[:, q, c] = mask_c * e
  * numer+denom, two subchunks per matmul: 16 accumulating matmuls with
    the TINY stationary pair [A_2q | A_2q+1] [128e, 4] (4-cycle weight
    load) streaming both subchunks' xb2 rows at N=258 into one [4, 258]
    psum tile; within each 129-column half, cols 0..127 = sum_e
    A[e,c]*x[e,:] and col 128 = sum_e A[e,c] (the masked exp sum = the
    softmax denominator).  Rows 0-1 x cols 0:129 hold the even subchunks'
    sums, rows 2-3 x cols 129:258 the odd subchunks' (the other quadrants
    are cross-terms, never read).
  * host folds the [4, G, 258] partials into numer [S, H] / denom [S].

A supermacro (4096 edges) can span at most SEGC=2 distinct segments
whenever every segment has >4096 edges (true for the target distribution);
the host verifies this and falls back to a pure-numpy path otherwise.
"""

import sys

sys.path.insert(0, "/opt/trn_rl_repo")

import ml_dtypes
import numpy as np

import concourse.bass as bass
import concourse.mybir as mybir
import concourse.tile as tile
from concourse.bass_utils import run_bass_kernel_spmd

NCORES = 8
H = 128
SUB = 128  # edges per subchunk (scores-matmul stationary width)
QPG = 32  # subchunks per supermacro
SUPER = SUB * QPG  # 4096
NSEG = 256
SEGC = 2  # segment columns per supermacro (max distinct segments)
RW = H + 1  # numer rhs width per subchunk (x row + ones col for denom)
BUFS = {"xb": 4, "xt": 3, "mk": 3, "feat_ps": 3, "fr": 3,
        "sc_ps": 2, "e": 2, "amat": 3, "nm_ps": 2}

BF16 = mybir.dt.bfloat16
F8E4 = mybir.dt.float8e4
F32 = mybir.dt.float32
AF = mybir.ActivationFunctionType
ALU = mybir.AluOpType


def _split_multi_waits(nc):
    """The walrus build in this container encodes at most one sync-wait per
    instruction; Tile emits several.  Spill extras onto standalone
    EventSemaphore instructions just before the gated instruction (same
    engine, so semantics are identical)."""
    for f in nc.m.functions:
        for b in f.blocks:
            insts = list(b.instructions)
            out = []
            changed = False
            for ins in insts:
                si = ins.sync_info
                waits = list(si.on_wait) if si else []
                if len(waits) > 1:
                    for k, w in enumerate(waits[1:]):
                        out.append(
                            mybir.InstEventSemaphore(
                                name=f"{ins.name}-wsplit{k}",
                                engine=ins.engine,
                                ins=[],
                                outs=[],
                                sync_info=mybir.SyncInfo(on_wait=[w], on_update=[]),
                            )
                        )
                    si.on_wait = waits[:1]
                    ins.sync_info = si
                    changed = True
                out.append(ins)
            if changed:
                b.instructions = out


def _build(G, b2_val, reps=1, stages=4, depth=1):
    """Build the single-core Bass program (shared verbatim by all 8 cores).

    reps>1 wraps the body in an on-device For_i loop re-running the whole
    kernel (same data) — used only for wall-clock benchmarking.
    stages<4 truncates the pipeline (1=DMA only, 2=+feats/relu,
    3=+scores/exp/amat, 4=full) — bench-only, output is garbage.
    depth: how many supermacros the numer stage lags the feats stage."""
    nc = bass.Bass()

    xb_d = nc.declare_dram_parameter("xb", [G, 128, QPG * RW], BF16, isOutput=False)
    xt_d = nc.declare_dram_parameter("xt", [G, 128, QPG * SUB], F8E4, isOutput=False)
    mk_d = nc.declare_dram_parameter("mk", [G, 128, QPG, SEGC], BF16, isOutput=False)
    w1t_d = nc.declare_dram_parameter("w1t", [H, H], BF16, isOutput=False)
    w2_d = nc.declare_dram_parameter("w2c", [H, 1], BF16, isOutput=False)
    b1_d = nc.declare_dram_parameter("b1c", [H, 1], F32, isOutput=False)
    out_d = nc.declare_dram_parameter(
        "partials", [2 * SEGC, G * 2 * RW], F32, isOutput=True
    )

    with tile.TileContext(nc) as tc:
        with (
            tc.tile_pool(name="consts", bufs=1) as cpool,
            tc.tile_pool(name="xb", bufs=BUFS["xb"]) as xpool,
            tc.tile_pool(name="xt", bufs=BUFS["xt"]) as tpool,
            tc.tile_pool(name="mk", bufs=BUFS["mk"]) as mpool,
            tc.tile_pool(name="feat_ps", bufs=BUFS["feat_ps"], space="PSUM") as fp_ps,
            tc.tile_pool(name="fr", bufs=BUFS["fr"]) as fr_pool,
            tc.tile_pool(name="sc_ps", bufs=BUFS["sc_ps"], space="PSUM") as sc_ps,
            tc.tile_pool(name="e", bufs=BUFS["e"]) as e_pool,
            tc.tile_pool(name="amat", bufs=BUFS["amat"]) as a_pool,
            tc.tile_pool(name="nm_ps", bufs=BUFS["nm_ps"], space="PSUM") as nm_ps,
            tc.tile_pool(name="stage", bufs=1) as st_pool,
        ):
            w1t = cpool.tile([H, H], BF16, name="w1t")
            nc.sync.dma_start(out=w1t[:], in_=w1t_d[:])
            w2c = cpool.tile([H, 1], BF16, name="w2c")
            nc.sync.dma_start(out=w2c[:], in_=w2_d[:])
            b1c = cpool.tile([H, 1], F32, name="b1c")
            nc.sync.dma_start(out=b1c[:], in_=b1_d[:])
            b2c = cpool.tile([128, 1], F32, name="b2c")
            nc.gpsimd.memset(b2c[:], b2_val)

            stage = st_pool.tile([2 * SEGC, G * 2 * RW], F32, name="stage")
            if stages < 4:  # bench-only variants never write stage
                nc.gpsimd.memset(stage[:], 0.0)

            import contextlib

            rep_ctx = tc.For_i(0, reps, 1) if reps > 1 else contextlib.nullcontext()
            with rep_ctx:
                _emit_body(
                    nc, tc, G, xb_d, xt_d, mk_d, w1t, w2c, b1c, b2c, stage,
                    xpool, tpool, mpool, fp_ps, fr_pool, sc_ps, e_pool, a_pool,
                    nm_ps, stages, depth,
                )

            nc.sync.dma_start(out=out_d[:], in_=stage[:])

    _split_multi_waits(nc)
    return nc


def _emit_body(
    nc, tc, G, xb_d, xt_d, mk_d, w1t, w2c, b1c, b2c, stage,
    xpool, tpool, mpool, fp_ps, fr_pool, sc_ps, e_pool, a_pool, nm_ps,
    stages=4, depth=1,
):
    # Software pipeline: iteration g runs feats(g) | scores(g-1) |
    # numer(g-1-depth+1) so PE doesn't wait on the ACT exp -> DVE amat
    # chain.
    prev = None  # (xb, fr, mk) of supermacro g-1
    prev2 = None  # (xb, amat) of supermacro g-2 (depth 2 only)
    for g in range(G + 1 + (depth - 1)):
        cur = None
        if g < G:
            xb = xpool.tile([128, QPG * RW], BF16, name="xb", tag="xb")
            nc.sync.dma_start(out=xb[:], in_=xb_d[g])
            # xt stays on the compute-free SP (sync) queue: any DMA config
            # placed on a compute engine's in-order queue (measured on both
            # the stripped and full pipeline) stalls prefetch behind that
            # engine's work and costs 30-40us end to end.
            xt = tpool.tile([128, QPG * SUB], F8E4, name="xt", tag="xt")
            nc.sync.dma_start(out=xt[:], in_=xt_d[g])
            mk = mpool.tile([128, QPG, SEGC], BF16, name="mk", tag="mk")
            nc.sync.dma_start(out=mk[:], in_=mk_d[g])
            if stages < 2:
                continue

            # feats.T = W1 @ x.T streamed straight from the host-transposed
            # layout; relu(+b1) lands in fr as bf16 for the scores matmuls.
            fr = fr_pool.tile([128, QPG * SUB], BF16, name="fr", tag="fr")
            for j in range(QPG * SUB // 512):
                fps = fp_ps.tile([128, 512], F32, name="fps", tag="fps")
                nc.tensor.matmul(
                    fps[:],
                    w1t[:],
                    xt[:, j * 512 : (j + 1) * 512],
                    start=True,
                    stop=True,
                )
                nc.scalar.activation(
                    fr[:, j * 512 : (j + 1) * 512], fps[:], AF.Relu,
                    bias=b1c[:, 0:1], scale=1.0,
                )
            cur = (xb, mk, fr)

        nxt2 = None
        if prev is not None and stages >= 3:
            xb, mk, fr = prev
            scps = sc_ps.tile([128, QPG], F32, name="scps", tag="scps")
            for q in range(QPG):
                nc.tensor.matmul(
                    scps[:, q : q + 1],
                    fr[:, q * SUB : (q + 1) * SUB],
                    w2c[:],
                    start=True,
                    stop=True,
                )
            e_sb = e_pool.tile([128, QPG], BF16, name="e_sb", tag="e_sb")
            nc.scalar.activation(
                e_sb[:], scps[:], AF.Exp, bias=b2c[:, 0:1], scale=1.0
            )
            amat = a_pool.tile([128, QPG, SEGC], BF16, name="amat", tag="amat")
            for cc in range(SEGC):
                nc.vector.tensor_mul(amat[:, :, cc], mk[:, :, cc], e_sb[:])
            nxt2 = (xb, amat)

        # numer+denom, two subchunks per matmul: the stationary is the pair
        # of A panels [128e, 2x2] (4-cycle load), the stream is both
        # subchunks' xb rows + ones columns at N=258.  psum rows 0-1 x cols
        # 0:129 accumulate the even subchunks' [c, h|denom] sums, rows 2-3
        # x cols 129:258 the odd subchunks' (the other two quadrants are
        # cross-term garbage, never read); the host adds the halves.
        if depth == 1:
            todo = (g - 1, nxt2)
        else:
            todo = (g - 2, prev2)
        if todo[1] is not None and stages >= 4:
            gp, (xb, amat) = todo
            nmps = nm_ps.tile([2 * SEGC, 2 * RW], F32, name="nmps", tag="nmps")
            for qq in range(QPG // 2):
                nc.tensor.matmul(
                    nmps[:],
                    amat[:, 2 * qq : 2 * qq + 2, :],
                    xb[:, 2 * qq * RW : (2 * qq + 2) * RW],
                    start=(qq == 0),
                    stop=(qq == QPG // 2 - 1),
                )
            nc.vector.tensor_copy(
                stage[:, gp * 2 * RW : (gp + 1) * 2 * RW], nmps[:]
            )

        prev2 = nxt2
        prev = cur


_prog_cache = {}


def _get_prog(G, b2_val):
    key = (G, float(b2_val))
    if key not in _prog_cache:
        _prog_cache[key] = _build(G, float(b2_val))
    return _prog_cache[key]


def _numpy_fallback(x, bi, W1, b1, w2, b2):
    feats = np.maximum(x @ W1.T + b1, 0)
    scores = feats @ w2 + float(b2)
    smax = scores.max() if scores.size else 0.0
    e = np.exp(scores - smax)
    off = np.searchsorted(bi, np.arange(NSEG + 1)).astype(np.int64)
    numer = np.zeros((NSEG, x.shape[1]), np.float32)
    denom = np.zeros(NSEG, np.float32)
    idx = np.minimum(off[:-1], max(len(bi) - 1, 0))
    if len(bi):
        r = np.add.reduceat(x * e[:, None], idx, axis=0)
        d = np.add.reduceat(e, idx)
        empty = off[:-1] == off[1:]
        r[empty] = 0
        d[empty] = 0
        numer[:] = r
        denom[:] = d
    out = np.zeros_like(numer)
    np.divide(numer, denom[:, None], out=out, where=denom[:, None] != 0)
    return out.astype(np.float32)


def prep_inputs(x, bi, W1, b1, w2):
    """Host-side prep: returns (in_maps, s0, G) or None if the data
    doesn't fit the device layout (caller falls back to numpy)."""
    E, Hdim = x.shape
    if Hdim != H or E % NCORES != 0 or E == 0:
        return None
    epc = E // NCORES
    G = -(-epc // SUPER)
    padded = G * SUPER

    seg = bi
    s0 = np.empty((NCORES, G), np.int64)
    ok = True
    for c in range(NCORES):
        sc = seg[c * epc : (c + 1) * epc]
        for g in range(G):
            lo = g * SUPER
            hi = min(lo + SUPER, epc)
            s0[c, g] = sc[lo]
            if sc[hi - 1] - sc[lo] > SEGC - 1:
                ok = False
    if not ok or np.any(np.diff(seg) < 0) or seg.min() < 0 or seg.max() >= NSEG:
        return None

    xb = x.astype(ml_dtypes.bfloat16)
    w1t_h = np.ascontiguousarray(W1.T).astype(ml_dtypes.bfloat16)
    w2_h = np.ascontiguousarray(w2[:, None]).astype(ml_dtypes.bfloat16)
    b1_h = np.ascontiguousarray(b1[:, None])

    in_maps = []
    for c in range(NCORES):
        xc = xb[c * epc : (c + 1) * epc]
        if padded != epc:
            xc = np.concatenate(
                [xc, np.zeros((padded - epc, H), ml_dtypes.bfloat16)], axis=0
            )
        # edge index = g*SUPER + p*QPG + q  ->  [G, 128, QPG, H]
        x4 = xc.reshape(G, 128, QPG, H)
        xb2 = np.ones((G, 128, QPG, RW), ml_dtypes.bfloat16)
        xb2[..., :H] = x4
        xb2 = np.ascontiguousarray(xb2.reshape(G, 128, QPG * RW))
        # transposed layout: [g, h, q, p] with e_local = q*128 + p; fp8 is
        # enough fidelity for the attention-score path (not for xb, whose
        # values reach the output directly).
        xt = np.ascontiguousarray(
            x4.transpose(0, 3, 2, 1).reshape(G, H, QPG * SUB)
        ).astype(ml_dtypes.float8_e4m3)

        sc = seg[c * epc : (c + 1) * epc]
        loc = np.full(padded, -1, np.int64)
        loc[:epc] = sc - np.repeat(s0[c], SUPER)[:epc]
        loc = loc.reshape(G, 128, QPG)
        mk = np.stack(
            [(loc == cc) for cc in range(SEGC)], axis=-1
        ).astype(ml_dtypes.bfloat16)

        in_maps.append(
            {
                "xb": xb2,
                "xt": xt,
                "mk": np.ascontiguousarray(mk),
                "w1t": w1t_h,
                "w2c": w2_h,
                "b1c": b1_h,
            }
        )
    return in_maps, s0, G


def kernel(x, batch_indices, W1, b1, w2, b2, _profile_sink=None):
    x = np.ascontiguousarray(np.asarray(x), dtype=np.float32)
    bi = np.asarray(batch_indices).astype(np.int64)
    W1 = np.asarray(W1, dtype=np.float32)
    b1 = np.asarray(b1, dtype=np.float32)
    w2 = np.asarray(w2, dtype=np.float32)
    b2f = float(np.asarray(b2))

    prep = prep_inputs(x, bi, W1, b1, w2)
    if prep is None:
        return _numpy_fallback(x, bi, W1, b1, w2, b2f)
    in_maps, s0, G = prep

    nc = _get_prog(G, b2f)
    res = run_bass_kernel_spmd(
        nc,
        in_maps,
        core_ids=list(range(NCORES)),
        **(_profile_sink if _profile_sink else {}),
    )
    if _profile_sink is not None:
        _profile_sink["results"] = res

    numer = np.zeros((NSEG, H), np.float64)
    denom = np.zeros(NSEG, np.float64)
    for c in range(NCORES):
        part4 = res.results[c]["partials"].reshape(2 * SEGC, G, 2 * RW)
        # even subchunks' sums live in rows 0:2 x cols 0:RW, odd subchunks'
        # in rows 2:4 x cols RW:2*RW (other quadrants are cross-terms)
        part = (
            part4[0:SEGC, :, 0:RW].astype(np.float64)
            + part4[SEGC : 2 * SEGC, :, RW : 2 * RW]
        )
        for cc in range(SEGC):
            segs = s0[c] + cc
            valid = segs < NSEG
            np.add.at(numer, segs[valid], part[cc][valid][:, :H])
            np.add.at(denom, segs[valid], part[cc][valid][:, H])

    out = np.zeros((NSEG, H), np.float32)
    np.divide(
        numer, denom[:, None], out=out, where=denom[:, None] != 0, casting="unsafe"
    )
    return out.astype(np.float32)
